# revision 58
# baseline (speedup 1.0000x reference)
"""Trainium2 Bass kernel for AdaptivePositionAwareAttention.

The reference multiplies dense [B,S,S] attention scores by a distance decay
pos_w = exp(-|i-j|/2).  In fp32, exp(final)-1 == 0 for |i-j| beyond ~90, so
the softmax collapses to a banded kernel plus a closed-form far field:

    out_i = (Vsum + sum_band (e^{f_ij}-1) V_j) / (S + sum_band (e^{f_ij}-1))

with Vsum = (sum_j xp_j) Wv^T + S bv.  The task/content/fusion branches
reduce to a per-row scalar g with final = base * g.

Sharding: 8 cores = (batch 0..3) x (sequence half); each core owns 1024 query
rows plus one 128-row zero-padded halo block per side.  Sequence-edge effects
are handled by per-core masked pos_w *data*, so all cores run one SPMD graph.
Activations are produced feature-major via PE transposes of xp; heavy matmuls
run as float32r (1 cyc/row; producers write f32r so walrus sees them rounded),
PE transposes stay fp32.  The global xp row-sum (for Vsum) is built from
per-block ones-vector matmul reductions plus one 3KB pair-wise AllReduce.
Phase 2 runs as a pipelined block-pair loop: scores -> per-row fusion ->
exp -> union-window E'^T -> PV matmul -> output projection.
"""

import math

import numpy as np

import concourse.bass as bass
from concourse import bacc, mybir, tile
from concourse.bass_utils import run_bass_kernel_spmd

B, S, H = 4, 2048, 768
HC = H // 128            # 6 feature chunks
NB = 10                  # halo blocks per core (1280 rows)
NO = 8                   # own blocks per core (1024 rows)
WIN = 384                # key window = 3 blocks
F32 = mybir.dt.float32
F32R = mybir.dt.float32r
AF = mybir.ActivationFunctionType
ALU = mybir.AluOpType
AX = mybir.AxisListType

_cache = {}


def f32(ap):
    """view a float32r AP as plain float32 (for PE transposes)."""
    return ap.bitcast(F32)


def build_kernel(dbg=False):
    nc = bacc.Bacc(None, target_bir_lowering=False)

    def din(name, shape, dt=F32):
        return nc.dram_tensor(name, shape, dt, kind="ExternalInput")

    xh = din("xh", [NB * 128, H])   # x halo slice, zero-padded
    ph = din("ph", [NB * 128, H])   # pos_emb halo slice, zero-padded
    wq = din("wq", [H, H], F32R)          # Wq^T  [hi, ho]
    wk = din("wk", [H, H], F32R)
    wv = din("wv", [H, H], F32R)
    wo = din("wo", [H, H], F32R)
    wc1 = din("wc1", [H, 384], F32R)      # Wc1^T
    wc2 = din("wc2", [384, 256], F32R)    # Wc2^T zero-padded 192->256
    wc3 = din("wc3", [256, 1], F32R)      # Wc3^T zero-padded
    wt1 = din("wt1", [64, H])       # Wt1^T
    wt2 = din("wt2", [H, H])        # Wt2^T
    wf1 = din("wf1", [3, H], F32R)        # Wf1^T / S
    wf2 = din("wf2", [H, 1], F32R)        # Wf2^T
    bqc = din("bqc", [128, HC])           # biases column-chunked
    bkc = din("bkc", [128, HC])
    bc1c = din("bc1c", [128, 3])
    bc2c = din("bc2c", [128, 2])
    bt1c = din("bt1c", [128, HC])
    bt2c = din("bt2c", [128, HC])
    bf1c = din("bf1c", [128, HC])
    bv_r = din("bv_r", [1, H])
    bo_r = din("bo_r", [1, H])
    bc3s = din("bc3s", [1, 1])
    bf2s = din("bf2s", [1, 1])
    te = din("te", [64, 1])         # task_table[task_id[b]]
    pw_f = din("pw_f", [128, WIN])        # pos_w tiles (1/sqrt(H) folded)
    pw_m = din("pw_m", [128, WIN])
    pw_l = din("pw_l", [128, WIN])
    ident = din("ident", [128, 128])
    ones_c = din("ones_c", [128, 1])
    ones_r = din("ones_r", [1, 128], F32R)

    y = nc.dram_tensor("y", [1024, H], F32, kind="ExternalOutput")
    if dbg:
        d_ts = nc.dram_tensor("d_ts", [1, 1], F32, kind="ExternalOutput")
        d_ci = nc.dram_tensor("d_ci", [1, 1024], F32, kind="ExternalOutput")
        d_m = nc.dram_tensor("d_m", [1, 1024], F32, kind="ExternalOutput")
        d_g8 = nc.dram_tensor("d_g8", [128, NO], F32, kind="ExternalOutput")
        d_zr8 = nc.dram_tensor("d_zr8", [128, NO], F32, kind="ExternalOutput")
        d_xs = nc.dram_tensor("d_xs", [128, HC], F32, kind="ExternalOutput")
        d_vs = nc.dram_tensor("d_vs", [1, H], F32, kind="ExternalOutput")
        d_q = nc.dram_tensor("d_q", [128, 128], F32, kind="ExternalOutput")
        d_k = nc.dram_tensor("d_k", [128, 128], F32, kind="ExternalOutput")
        d_v = nc.dram_tensor("d_v", [128, H], F32, kind="ExternalOutput")
        d_b1 = nc.dram_tensor("d_b1", [128, WIN], F32, kind="ExternalOutput")
        d_xpt = nc.dram_tensor("d_xpt", [128, 128], F32, kind="ExternalOutput")

    with tile.TileContext(nc) as tc:
        with (
            tc.tile_pool(name="pers", bufs=1) as pers,
            tc.tile_pool(name="ptr", bufs=2, space="PSUM") as ptr,
        ):
            idn = pers.tile([128, 128], F32, tag="idn")
            nc.sync.dma_start(idn[:], ident[:])
            onc = pers.tile([128, 1], F32, tag="onc")
            nc.sync.dma_start(onc[:], ones_c[:])
            onr = pers.tile([1, 128], F32R, tag="onr")
            nc.sync.dma_start(onr[:], ones_r[:])
            bo_r8 = pers.tile([1, H], F32R, tag="bor8")

            # --- bias / small tiles -------------------------------------
            bq_t = pers.tile([128, HC], F32, tag="bq")
            nc.sync.dma_start(bq_t[:], bqc[:])
            bk_t = pers.tile([128, HC], F32, tag="bk")
            nc.sync.dma_start(bk_t[:], bkc[:])
            bc1_t = pers.tile([128, 3], F32, tag="bc1")
            nc.sync.dma_start(bc1_t[:], bc1c[:])
            bc2_t = pers.tile([128, 2], F32, tag="bc2")
            nc.sync.dma_start(bc2_t[:], bc2c[:])
            bf1_t = pers.tile([128, HC], F32, tag="bf1")
            nc.sync.dma_start(bf1_t[:], bf1c[:])
            bv_t = pers.tile([1, H], F32, tag="bvr")
            nc.sync.dma_start(bv_t[:], bv_r[:])
            bo_t = pers.tile([1, H], F32, tag="bor")
            nc.sync.dma_start(bo_t[:], bo_r[:])
            bc3_t = pers.tile([1, 1], F32, tag="bc3")
            nc.sync.dma_start(bc3_t[:], bc3s[:])
            bf2_t = pers.tile([1, 1], F32, tag="bf2")
            nc.sync.dma_start(bf2_t[:], bf2s[:])

            bv_b = pers.tile([128, H], F32, tag="bvb")
            nc.gpsimd.partition_broadcast(bv_b[:], bv_t[:])
            nc.vector.tensor_copy(bo_r8[:], bo_t[:])

            # --- task scalar ts (tiny, scoped) -------------------------
            ts_t = pers.tile([1, 1], F32, tag="ts")
            ts1_t = pers.tile([1, 1], F32, tag="ts1")
            with (
                tc.tile_pool(name="tsk", bufs=1) as tsk,
                tc.tile_pool(name="ptsk", bufs=2, space="PSUM") as ptsk,
            ):
                te_t = tsk.tile([64, 1], F32, tag="te")
                nc.gpsimd.dma_start(te_t[:], te[:])
                bt1_t = tsk.tile([128, HC], F32, tag="bt1")
                nc.sync.dma_start(bt1_t[:], bt1c[:])
                bt2_t = tsk.tile([128, HC], F32, tag="bt2")
                nc.sync.dma_start(bt2_t[:], bt2c[:])
                wt1_t = tsk.tile([64, H], F32, tag="wt1")
                nc.gpsimd.dma_start(wt1_t[:], wt1[:])
                wt2_t = tsk.tile([128, HC * H], F32, tag="wt2")
                for c in range(HC):
                    nc.gpsimd.dma_start(wt2_t[:, c * H:(c + 1) * H],
                                        wt2[c * 128:(c + 1) * 128, :])
                t1_t = tsk.tile([128, HC], F32, tag="t1")
                for c in range(HC):
                    p = ptsk.tile([128, 1], F32, tag="tp")
                    nc.tensor.matmul(p[:], wt1_t[:, c * 128:(c + 1) * 128],
                                     te_t[:], start=True, stop=True)
                    nc.scalar.activation(t1_t[:, c:c + 1], p[:], AF.Relu,
                                         bias=bt1_t[:, c:c + 1])
                tw_t = tsk.tile([128, HC], F32, tag="tw")
                for c in range(HC):
                    p = ptsk.tile([128, 1], F32, tag="tp")
                    for ci_ in range(HC):
                        nc.tensor.matmul(
                            p[:],
                            wt2_t[:, ci_ * H + c * 128: ci_ * H + (c + 1) * 128],
                            t1_t[:, ci_:ci_ + 1],
                            start=(ci_ == 0), stop=(ci_ == HC - 1))
                    nc.scalar.activation(tw_t[:, c:c + 1], p[:], AF.Sigmoid,
                                         bias=bt2_t[:, c:c + 1])
                tws = tsk.tile([128, 1], F32, tag="tws")
                nc.vector.tensor_reduce(tws[:], tw_t[:], AX.X, ALU.add)
                pts = ptsk.tile([1, 1], F32, tag="tsp")
                nc.tensor.matmul(pts[:], tws[:], onc[:], start=True, stop=True)
                nc.vector.tensor_scalar_mul(ts_t[:], pts[:], 1.0 / H)
                nc.vector.tensor_scalar(ts1_t[:], ts_t[:], -0.5, 1.0,
                                        ALU.mult, ALU.add)

            # --- phase 1: projections ----------------------------------
            qkpool = tc.tile_pool(name="qkp", bufs=1)
            qkp = qkpool.__enter__()
            qT = qkp.tile([128, HC * 1024], F32R, tag="qT")
            kT = qkp.tile([128, HC * NB * 128], F32R, tag="kT")
            vT = pers.tile([128, NB * H], F32R, tag="v")    # row-major V
            vsumc = pers.tile([128, HC], F32, tag="vsumc")

            h1pool = tc.tile_pool(name="h1p", bufs=1)
            h1pp = h1pool.__enter__()
            with (
                tc.tile_pool(name="cnt", bufs=1) as cnt,
                tc.tile_pool(name="wstr", bufs=2) as wstr,
                tc.tile_pool(name="xpl", bufs=3) as xpl,
                tc.tile_pool(name="xpt", bufs=1) as xptp,
                tc.tile_pool(name="pqk", bufs=3, space="PSUM") as pqk,
                tc.tile_pool(name="pv", bufs=1, space="PSUM") as pv,
                tc.tile_pool(name="pxs", bufs=1, space="PSUM") as pxs,
            ):
                h1T = h1pp.tile([128, 3 * 1024], F32R, tag="h1T")
                xsum_sb = cnt.tile([128, HC], F32, tag="xsumsb")

                # load + transpose xp (x + pos) into xpT feature-major
                xpT = xptp.tile([128, HC * NB * 128], F32R, tag="xpT")
                for u in range(NB):
                    xp_u = xpl.tile([128, H], F32, tag="xp")
                    nc.sync.dma_start(xp_u[:], xh[u * 128:(u + 1) * 128, :])
                    nc.gpsimd.dma_start(xp_u[:], ph[u * 128:(u + 1) * 128, :],
                                        accum_op=ALU.add)
                    if 1 <= u <= 8:
                        pp = pxs.tile([128, HC], F32, tag="xsum")
                        for c in range(HC):
                            nc.tensor.matmul(
                                pp[:, c:c + 1],
                                xp_u[:, c * 128:(c + 1) * 128], onc[:],
                                start=True, stop=True)
                        if u == 1:
                            nc.vector.tensor_copy(xsum_sb[:], pp[:])
                        else:
                            nc.vector.tensor_tensor(xsum_sb[:], xsum_sb[:],
                                                    pp[:], ALU.add)
                    for c in range(HC):
                        p = ptr.tile([128, 128], F32, tag="trp")
                        nc.tensor.transpose(p[:],
                                            xp_u[:, c * 128:(c + 1) * 128],
                                            idn[:])
                        nc.scalar.copy(
                            xpT[:, (c * NB + u) * 128:(c * NB + u + 1) * 128],
                            p[:])

                def load_w(dram, wstride):
                    """stream a [768, wstride] weight as two half tiles."""
                    halves = []
                    for hh in range(2):
                        wt = wstr.tile([128, 3 * H], F32R, tag="w")
                        for c in range(3):
                            cc = hh * 3 + c
                            nc.scalar.dma_start(
                                wt[:, c * wstride:(c + 1) * wstride],
                                dram[cc * 128:(cc + 1) * 128, :])
                        halves.append(wt)
                    return halves

                def proj_pairs(halves, wstride, out, mode, bias_t,
                               n_out_chunks, own_only, out_stride):
                    """out[oc*stride + s] = sum_hi w[hi,oc]^T xpT[hi,s] (+b)."""
                    for pr in range(5):       # 256-col pairs over 1280
                        s0 = pr * 256
                        if own_only and (s0 + 256 <= 128 or s0 >= 1152):
                            continue
                        for oc in range(n_out_chunks):
                            p = pqk.tile([128, 256], F32, tag="pqk")
                            for c in range(HC):
                                wtile = halves[c // 3]
                                cc = c % 3
                                nc.tensor.matmul(
                                    p[:],
                                    wtile[:, cc * wstride + oc * 128:
                                          cc * wstride + oc * 128 + 128],
                                    xpT[:, c * NB * 128 + s0:
                                        c * NB * 128 + s0 + 256],
                                    start=(c == 0), stop=(c == HC - 1))
                            if own_only:
                                o0, o1 = max(s0, 128), min(s0 + 256, 1152)
                                dst = out[:, oc * out_stride + o0 - 128:
                                          oc * out_stride + o1 - 128]
                                src = p[:, o0 - s0:o1 - s0]
                            else:
                                dst = out[:, oc * out_stride + s0:
                                          oc * out_stride + s0 + 256]
                                src = p[:]
                            if mode == "copy":
                                nc.vector.tensor_copy(dst, src)
                            elif mode == "relu":
                                nc.scalar.activation(dst, src, AF.Relu,
                                                     bias=bias_t[:, oc:oc + 1])
                            elif mode == "abias":
                                nc.scalar.activation(dst, src, AF.Identity,
                                                     bias=bias_t[:, oc:oc + 1])
                            else:
                                nc.vector.tensor_scalar_add(dst, src,
                                                            bias_t[:, oc:oc + 1])

                # K^T feature-major, with bias
                wh = load_w(wk, H)
                proj_pairs(wh, H, kT, "bias", bk_t, HC, False, NB * 128)

                # Q^T feature-major (own rows), with bias
                wh = load_w(wq, H)
                proj_pairs(wh, H, qT, "abias", bq_t, HC, True, 1024)

                # content h1T (own rows), relu+bias
                wh = load_w(wc1, 384)
                proj_pairs(wh, 384, h1T, "relu", bc1_t, 3, True, 1024)
                # V row-major, with bias via bcast add
                wh = load_w(wv, H)
                for u in range(NB):
                    pvt = pv.tile([128, H], F32, tag="pv")
                    for c in range(HC):
                        w_t = wh[c // 3]
                        cw = (c % 3) * H
                        for n0, n1 in ((0, 512), (512, 768)):
                            nc.tensor.matmul(
                                pvt[:, n0:n1],
                                xpT[:, (c * NB + u) * 128:(c * NB + u + 1) * 128],
                                w_t[:, cw + n0: cw + n1],
                                start=(c == 0), stop=(c == HC - 1))
                    nc.vector.tensor_tensor(vT[:, u * H:(u + 1) * H], pvt[:],
                                            bv_b[:], ALU.add)

                # global xp sum: AllReduce own-half sums across the pair
                with tc.tile_pool(name="ccd", bufs=1, space="DRAM") as ccd:
                    cc_in = ccd.tile([128, HC], F32, tag="cci")
                    cc_out = ccd.tile([128, HC], F32, tag="cco")
                    nc.gpsimd.dma_start(cc_in[:], xsum_sb[:])
                    nc.gpsimd.collective_compute(
                        "AllReduce", ALU.add,
                        replica_groups=[[0, 1], [2, 3], [4, 5], [6, 7]],
                        ins=[cc_in[:].opt()], outs=[cc_out[:].opt()])
                    xsum_f = cnt.tile([128, HC], F32, tag="xsumf")
                    nc.gpsimd.dma_start(xsum_f[:], cc_out[:])
                xsum_s = cnt.tile([128, HC], F32R, tag="xsums")
                nc.vector.tensor_copy(xsum_s[:], xsum_f[:])
                if True:
                    for nh in range(2):
                        vs_p = pxs.tile([1, 384], F32, tag="xsum")
                        for c in range(HC):
                            nc.tensor.matmul(
                                vs_p[:],
                                xsum_s[:, c:c + 1],
                                wh[c // 3][:, (c % 3) * H + nh * 384:
                                           (c % 3) * H + (nh + 1) * 384],
                                start=(c == 0), stop=(c == HC - 1))
                        vs_h = cnt.tile([1, 384], F32, tag="vsh")
                        # vs_h = S*bv_half + vs_p  (fold the S*bv term here)
                        nc.vector.tensor_scalar_mul(
                            vs_h[:], bv_t[:, nh * 384:(nh + 1) * 384], float(S))
                        nc.vector.tensor_tensor(vs_h[:], vs_h[:], vs_p[:],
                                                ALU.add)
                        for c3 in range(3):
                            vtp = ptr.tile([128, 1], F32, tag="trp")
                            nc.tensor.transpose(
                                vtp[:], vs_h[:, c3 * 128:(c3 + 1) * 128],
                                idn[0:1, 0:1])
                            nc.vector.tensor_copy(
                                vsumc[:, nh * 3 + c3: nh * 3 + c3 + 1], vtp[:])


                if dbg:
                    nc.sync.dma_start(d_xs[:], f32(xsum_s[:]))
                    nc.sync.dma_start(d_q[:], f32(qT[:, 0:128]))
                    nc.sync.dma_start(d_k[:], f32(kT[:, 128:256]))
                    nc.sync.dma_start(d_v[:], f32(vT[:, H:2 * H]))
                    nc.sync.dma_start(d_xpt[:], f32(xpT[:, 128:256]))

            if False and dbg:
                nc.sync.dma_start(d_xs[:], f32(xsum_s[:]))
                nc.sync.dma_start(d_vs[:], vsum_b[0:1, :])
                nc.sync.dma_start(d_q[:], f32(qT[:, 0:128]))
                nc.sync.dma_start(d_k[:], f32(kT[:, 128:256]))
                nc.sync.dma_start(d_v[:], f32(vT[:, H:2 * H]))
                nc.sync.dma_start(d_xpt[:], f32(xpT[:, 128:256]))

            # --- content tail: h2T, ci (after xpT/weights freed) -------
            ci_r = pers.tile([1, 1024], F32, tag="cir")
            with (
                tc.tile_pool(name="cnt2", bufs=1) as cnt2,
                tc.tile_pool(name="pqk2", bufs=2, space="PSUM") as pqk2,
            ):
                h2T = cnt2.tile([128, 2 * 1024], F32R, tag="h2T")
                w2_t = cnt2.tile([128, 3 * 256], F32R, tag="w2")
                for c in range(3):
                    nc.sync.dma_start(w2_t[:, c * 256:(c + 1) * 256],
                                      wc2[c * 128:(c + 1) * 128, :])
                for oc in range(2):
                    for nh in range(2):
                        p = pqk2.tile([128, 512], F32, tag="pqk2")
                        for c in range(3):
                            nc.tensor.matmul(
                                p[:],
                                w2_t[:, c * 256 + oc * 128:
                                     c * 256 + oc * 128 + 128],
                                h1T[:, c * 1024 + nh * 512:
                                    c * 1024 + (nh + 1) * 512],
                                start=(c == 0), stop=(c == 2))
                        nc.scalar.activation(
                            h2T[:, oc * 1024 + nh * 512:
                                oc * 1024 + (nh + 1) * 512],
                            p[:], AF.Relu, bias=bc2_t[:, oc:oc + 1])
                w3_t = cnt2.tile([128, 2], F32R, tag="wc3")
                for c in range(2):
                    nc.sync.dma_start(w3_t[:, c:c + 1],
                                      wc3[c * 128:(c + 1) * 128, :])
                for nh in range(2):
                    ci_p = pqk2.tile([1, 512], F32, tag="cip")
                    for c in range(2):
                        nc.tensor.matmul(
                            ci_p[:],
                            w3_t[:, c:c + 1],
                            h2T[:, c * 1024 + nh * 512:
                                c * 1024 + (nh + 1) * 512],
                            start=(c == 0), stop=(c == 1))
                    nc.scalar.activation(ci_r[:, nh * 512:(nh + 1) * 512],
                                         ci_p[:], AF.Sigmoid, bias=bc3_t[:])

            h1pool.__exit__(None, None, None)
            if dbg:
                nc.sync.dma_start(d_ci[:], ci_r[:])
                nc.sync.dma_start(d_ts[:], ts_t[:])

            # --- phase 2: merged per-pair pipeline ----------------------
            bo_b = pers.tile([128, H], F32, tag="bob")
            nc.gpsimd.partition_broadcast(bo_b[:], bo_t[:])
            wo_t = pers.tile([128, HC * H], F32R, tag="wo")
            for c in range(HC):
                nc.scalar.dma_start(wo_t[:, c * H:(c + 1) * H],
                                    wo[c * 128:(c + 1) * 128, :])
            pw_t = {}
            for nm, d in (("f", pw_f), ("m", pw_m), ("l", pw_l)):
                t_ = pers.tile([128, WIN], F32, tag="pw" + nm)
                nc.sync.dma_start(t_[:], d[:])
                pw_t[nm] = t_
            wf1_t = pers.tile([3, H], F32R, tag="wf1")
            nc.sync.dma_start(wf1_t[:], wf1[:])
            wf2_t = pers.tile([128, HC], F32R, tag="wf2")
            for c in range(HC):
                nc.sync.dma_start(wf2_t[:, c:c + 1],
                                  wf2[c * 128:(c + 1) * 128, :])

            ztile = pers.tile([128, 128], F32, tag="ztile")
            nc.vector.memset(ztile[:], 0.0)
            g8 = pers.tile([128, NO], F32, tag="g8")
            zr8 = pers.tile([128, NO], F32, tag="zr8")
            ts_b = pers.tile([128, 1], F32, tag="tsb")
            nc.gpsimd.partition_broadcast(ts_b[:], ts_t[:])

            with (
                tc.tile_pool(name="wk2", bufs=2) as wk2,
                tc.tile_pool(name="wk3", bufs=4) as wk3,
                tc.tile_pool(name="bps8", bufs=8) as bps8,
                tc.tile_pool(name="psc", bufs=2, space="PSUM") as psc,
                tc.tile_pool(name="pny", bufs=2, space="PSUM") as pny,
            ):
                bp_all = {}
                for t0 in (1, 3, 5, 7):
                    fin_p = wk3.tile([3, 256], F32R, tag="finp")
                    for hh in range(2):
                        t = t0 + hh
                        sp = psc.tile([128, WIN], F32, tag="sc")
                        for c in range(HC):
                            nc.tensor.matmul(
                                sp[:],
                                qT[:, c * 1024 + (t - 1) * 128:
                                   c * 1024 + t * 128],
                                kT[:, (c * NB + t - 1) * 128:
                                   (c * NB + t + 2) * 128],
                                start=(c == 0), stop=(c == HC - 1))
                        pw = pw_t["f"] if t == 1 else (
                            pw_t["l"] if t == 8 else pw_t["m"])
                        bp = bps8.tile([128, WIN], F32, tag="bp")
                        nc.vector.tensor_tensor(bp[:], sp[:], pw[:], ALU.mult)
                        bp_all[t] = bp
                        # fusion inputs (column domain)
                        ftin = wk3.tile([128, 3], F32, tag="ftin")
                        nc.vector.tensor_reduce(ftin[:, 0:1], bp[:], AX.X,
                                                ALU.add)
                        cp = ptr.tile([128, 1], F32, tag="trp")
                        nc.tensor.transpose(cp[:],
                                            ci_r[:, (t - 1) * 128: t * 128],
                                            idn[0:1, 0:1])
                        cic = wk3.tile([128, 1], F32, tag="cic")
                        nc.vector.tensor_copy(cic[:], cp[:])
                        nc.vector.tensor_scalar_mul(ftin[:, 1:2], ftin[:, 0:1],
                                                    ts_b[:])
                        nc.vector.tensor_tensor(ftin[:, 2:3], ftin[:, 0:1],
                                                cic[:], ALU.mult)
                        fp = ptr.tile([3, 128], F32, tag="trp")
                        nc.tensor.transpose(fp[:], ftin[:], idn[:])
                        nc.vector.tensor_copy(
                            fin_p[:, hh * 128:(hh + 1) * 128], fp[:])
                    # pair fusion MLP (N=256)
                    fu1s = wk2.tile([128, HC * 256], F32R, tag="fu1s")
                    for c in range(HC):
                        fup = psc.tile([128, 256], F32, tag="sc")
                        nc.tensor.matmul(fup[:],
                                         wf1_t[:, c * 128:(c + 1) * 128],
                                         fin_p[:], start=True, stop=True)
                        nc.scalar.activation(
                            fu1s[:, c * 256:(c + 1) * 256], fup[:],
                            AF.Relu, bias=bf1_t[:, c:c + 1])
                    fwp = psc.tile([1, 256], F32, tag="sc")
                    for c in range(HC):
                        nc.tensor.matmul(fwp[:], wf2_t[:, c:c + 1],
                                         fu1s[:, c * 256:(c + 1) * 256],
                                         start=(c == 0), stop=(c == HC - 1))
                    fw_s = wk3.tile([1, 256], F32, tag="fws")
                    nc.scalar.activation(fw_s[:], fwp[:], AF.Sigmoid,
                                         bias=bf2_t[:])
                    q_s = wk3.tile([1, 256], F32, tag="qs")
                    nc.vector.tensor_scalar(
                        q_s[:], ci_r[:, (t0 - 1) * 128:(t0 + 1) * 128],
                        -0.5, ts1_t[:], ALU.mult, ALU.add)
                    nc.vector.tensor_tensor(q_s[:], fw_s[:], q_s[:], ALU.mult)
                    nc.vector.tensor_scalar(q_s[:], q_s[:], -1.0, 1.0,
                                            ALU.mult, ALU.add)
                    for hh in range(2):
                        t = t0 + hh
                        gp = ptr.tile([128, 1], F32, tag="trp")
                        nc.tensor.transpose(gp[:],
                                            q_s[:, hh * 128:(hh + 1) * 128],
                                            idn[0:1, 0:1])
                        nc.vector.tensor_copy(g8[:, t - 1:t], gp[:])
                for t0 in (1, 3, 5, 7):
                    es = []
                    for hh in range(2):
                        t = t0 + hh
                        e_t = wk3.tile([128, WIN], F32, tag="et")
                        zc = wk3.tile([128, 1], F32, tag="zc")
                        nc.scalar.activation(e_t[:], bp_all[t][:], AF.Exp,
                                             scale=g8[:, t - 1:t],
                                             accum_out=zc[:])
                        nc.vector.tensor_scalar_add(zc[:], zc[:],
                                                    float(S - WIN))
                        nc.vector.reciprocal(zr8[:, t - 1:t], zc[:])
                        es.append(e_t)
                    # union-window E'^T for the pair
                    etp = wk2.tile([128, 4 * 256], F32R, tag="etp")
                    nc.vector.tensor_copy(etp[:, 128:256], ztile[:])
                    nc.vector.tensor_copy(etp[:, 3 * 256: 3 * 256 + 128],
                                          ztile[:])
                    for hh in range(2):
                        for w in range(3):
                            p = ptr.tile([128, 128], F32, tag="trp")
                            nc.tensor.transpose(
                                p[:], es[hh][:, w * 128:(w + 1) * 128], idn[:])
                            nc.vector.tensor_scalar_sub(
                                etp[:, (w + hh) * 256 + hh * 128:
                                    (w + hh) * 256 + hh * 128 + 128],
                                p[:], 1.0)
                    # NUMT: NT[ho, i_pair] += V_u^T E'_u  (two c-halves)
                    nt_s = wk2.tile([128, HC * 256], F32R, tag="nts")
                    for ch in range(2):
                        ntp = pny.tile([128, 3 * 256], F32, tag="ny")
                        for c3 in range(3):
                            c = ch * 3 + c3
                            for q in range(4):
                                nc.tensor.matmul(
                                    ntp[:, c3 * 256:(c3 + 1) * 256],
                                    vT[:, (t0 - 1 + q) * H + c * 128:
                                       (t0 - 1 + q) * H + (c + 1) * 128],
                                    etp[:, q * 256:(q + 1) * 256],
                                    start=(q == 0), stop=(q == 3))
                        for c3 in range(3):
                            c = ch * 3 + c3
                            nc.vector.tensor_scalar_add(
                                nt_s[:, c * 256:(c + 1) * 256],
                                ntp[:, c3 * 256:(c3 + 1) * 256],
                                vsumc[:, c:c + 1])
                    for hh in range(2):
                        t = t0 + hh
                        yp = pny.tile([128, H], F32, tag="ny")
                        for n0, n1 in ((0, 512), (512, 768)):
                            for c in range(HC):
                                nc.tensor.matmul(
                                    yp[:, n0:n1],
                                    nt_s[:, c * 256 + hh * 128:
                                         c * 256 + hh * 128 + 128],
                                    wo_t[:, c * H + n0: c * H + n1],
                                    start=(c == 0), stop=(c == HC - 1))
                        y_t = wk3.tile([128, H], F32, tag="yt")
                        nc.vector.tensor_scalar_mul(y_t[:], yp[:],
                                                    zr8[:, t - 1:t])
                        nc.vector.tensor_tensor(y_t[:], y_t[:], bo_b[:],
                                                ALU.add)
                        nc.sync.dma_start(y[(t - 1) * 128: t * 128, :], y_t[:])
                if dbg:
                    nc.sync.dma_start(d_g8[:], g8[:])
                    nc.sync.dma_start(d_zr8[:], zr8[:])

            qkpool.__exit__(None, None, None)

    nc.compile()
    return nc


def prep_inputs(x, task_id, pos_emb, Wq, bq, Wk, bk, Wv, bv, Wo, bo,
                task_table, Wt1, bt1, Wt2, bt2,
                Wc1, bc1, Wc2, bc2, Wc3, bc3,
                Wf1, bf1, Wf2, bf2):
    f = np.float32
    T = lambda a: np.ascontiguousarray(np.asarray(a).T, dtype=f)
    C = lambda a: np.ascontiguousarray(np.asarray(a), dtype=f)
    shared = {
        "wq": T(Wq), "wk": T(Wk), "wv": T(Wv), "wo": T(Wo),
        "wc1": T(Wc1), "wt1": T(Wt1), "wt2": T(Wt2),
        "wf1": T(Wf1) / S, "wf2": T(Wf2),
        "bqc": C(np.asarray(bq).reshape(HC, 128).T),
        "bkc": C(np.asarray(bk).reshape(HC, 128).T),
        "bc1c": C(np.asarray(bc1).reshape(3, 128).T),
        "bt1c": C(np.asarray(bt1).reshape(HC, 128).T),
        "bt2c": C(np.asarray(bt2).reshape(HC, 128).T),
        "bf1c": C(np.asarray(bf1).reshape(HC, 128).T),
        "bv_r": C(np.asarray(bv).reshape(1, H)),
        "bo_r": C(np.asarray(bo).reshape(1, H)),
        "bc3s": C(np.asarray(bc3).reshape(1, 1)),
        "bf2s": C(np.asarray(bf2).reshape(1, 1)),
        "ident": np.eye(128, dtype=f),
        "ones_c": np.ones((128, 1), f),
        "ones_r": np.ones((1, 128), f),
    }
    wc2p = np.zeros((384, 256), f); wc2p[:, :192] = T(Wc2)
    wc3p = np.zeros((256, 1), f); wc3p[:192] = T(Wc3)
    bc2p = np.zeros(256, f); bc2p[:192] = np.asarray(bc2)
    shared["wc2"] = wc2p; shared["wc3"] = wc3p
    shared["bc2c"] = C(bc2p.reshape(2, 128).T)

    p_ = np.arange(128)[:, None]; c_ = np.arange(WIN)[None, :]
    pwm = (np.exp(-np.abs(128 + p_ - c_) / 2.0) / math.sqrt(H)).astype(f)

    x = np.asarray(x); pos_emb = np.asarray(pos_emb)
    task_table = np.asarray(task_table); task_id = np.asarray(task_id)
    in_maps = []
    for core in range(8):
        b, half = core // 2, core % 2
        g0 = 1024 * half - 128
        xhv = np.zeros((1280, H), f); phh = np.zeros((1280, H), f)
        lo, hi = max(0, g0), min(S, g0 + 1280)
        xhv[lo - g0:hi - g0] = x[b, lo:hi]
        phh[lo - g0:hi - g0] = pos_emb[0, lo:hi]
        pwf = pwm.copy(); pwl = pwm.copy()
        if half == 0:
            pwf[:, :128] = 0
        if half == 1:
            pwl[:, 256:] = 0
        m = dict(shared)
        m.update({
            "xh": xhv, "ph": phh,
            "te": C(task_table[int(task_id[b])].reshape(64, 1)),
            "pw_f": pwf, "pw_m": pwm, "pw_l": pwl,
        })
        in_maps.append(m)
    return in_maps


class _Runner:
    """Compile the SPMD graph once and keep a reusable jitted callable."""

    def __init__(self, nc, n_cores=8):
        import jax
        from jax.sharding import Mesh, PartitionSpec
        from jax.experimental.shard_map import shard_map
        from concourse import bass2jax, mybir as _mb
        bass2jax.install_neuronx_cc_hook()
        self.nc = nc
        partition_name = (nc.partition_id_tensor.name
                          if nc.partition_id_tensor else None)
        in_names, out_names, out_avals, zero_outs = [], [], [], []
        for alloc in nc.m.functions[0].allocations:
            if not isinstance(alloc, _mb.MemoryLocationSet):
                continue
            name = alloc.memorylocations[0].name
            if alloc.kind == "ExternalInput":
                if name != partition_name:
                    in_names.append(name)
            elif alloc.kind == "ExternalOutput":
                shape = tuple(alloc.tensor_shape)
                dtype = _mb.dt.np(alloc.dtype)
                out_names.append(name)
                out_avals.append(jax.core.ShapedArray(shape, dtype))
                zero_outs.append(np.zeros(shape, dtype))
        self.in_names = list(in_names)
        self.out_names = out_names
        self.out_avals = out_avals
        self.zero_outs = zero_outs
        self.n_cores = n_cores
        n_params = len(self.in_names)
        all_in = list(self.in_names) + list(out_names)
        if partition_name is not None:
            all_in.append(partition_name)

        def _body(*args):
            operands = list(args)
            if partition_name is not None:
                operands.append(bass2jax.partition_id_tensor())
            outs = bass2jax._bass_exec_p.bind(
                *operands,
                out_avals=tuple(out_avals),
                in_names=tuple(all_in),
                out_names=tuple(out_names),
                lowering_input_output_aliases=(),
                sim_require_finite=True,
                sim_require_nnan=True,
                nc=nc,
            )
            return tuple(outs)

        devices = jax.devices()[:n_cores]
        mesh = Mesh(np.asarray(devices), ("core",))
        n_outs = len(out_names)
        in_specs = (PartitionSpec("core"),) * (n_params + n_outs)
        out_specs = (PartitionSpec("core"),) * n_outs
        self.fn = jax.jit(
            shard_map(_body, mesh=mesh, in_specs=in_specs,
                      out_specs=out_specs, check_rep=False),
            keep_unused=True)

    def concat_inputs(self, in_maps):
        return [np.concatenate([np.asarray(in_maps[c][k])
                                for c in range(self.n_cores)], axis=0)
                for k in self.in_names]

    def zeros(self):
        return [np.zeros((self.n_cores * z.shape[0],) + z.shape[1:], z.dtype)
                for z in self.zero_outs]

    def __call__(self, concat_in, zeros=None):
        import jax
        if zeros is None:
            zeros = self.zeros()
        outs = jax.block_until_ready(self.fn(*concat_in, *zeros))
        return outs


def get_runner():
    if "runner" not in _cache:
        _cache["runner"] = _Runner(build_kernel())
    return _cache["runner"]


def kernel(**inputs):
    rn = get_runner()
    in_maps = prep_inputs(**inputs)
    outs = rn(rn.concat_inputs(in_maps))
    yc = np.asarray(outs[rn.out_names.index("y")]).reshape(8, 1024, H)
    out = np.zeros((B, S, H), np.float32)
    for core in range(8):
        b, half = core // 2, core % 2
        out[b, 1024 * half:1024 * (half + 1)] = yc[core]
    return out
